# revision 1
# baseline (speedup 1.0000x reference)
"""Trainium2 kernel for nn_LocalPatternExtractor (binary-weight depthwise+pointwise
conv -> BatchNorm -> quantized LIF over 4 timesteps).

Forward-pass analysis
---------------------
The reference quantizes the membrane potential with
    step = THRESHOLD / 2**(POT_BITS-1) = 1/128
    q    = clip(round(v/step), -128, 127) * step
so after quantization  mem <= 127/128 = 0.9921875 < THRESHOLD (=1.0), with
f32 STE round-off bounded by ~|v|*2^-24 << 1/128.  Hence `mem >= THRESHOLD`
is false for every element at every timestep, no spike ever fires, and the
forward output is identically
    out      = zeros((B, C_out, L), float32)
    reg_loss = SPIKE_REG * mean(out) = 0.0
for *all* finite inputs (verified empirically against the jax reference for
several seeds and 10x-scaled inputs).  The optimal kernel therefore reduces
to materializing the zero output at HBM write roofline.

Sharding: pure data parallel over the batch dim (16 -> 2 per core on 8
cores); each core zero-fills its own (2, 256, 5000) f32 output shard
(10.24 MB), which the host concatenates.
"""

import numpy as np

import concourse.bass as bass
import concourse.mybir as mybir
from concourse.bass_utils import run_bass_kernel_spmd

N_CORES = 8
B, C_IN, L = 16, 12, 5000
C_OUT = 256
K = 3

B_LOC = B // N_CORES               # 2 batches per core
OUT_ELEMS = B_LOC * C_OUT * L      # 2,560,000 f32 per core (10.24 MB)
P = 128                            # SBUF partitions
COLS = OUT_ELEMS // P              # 20,000 f32 per partition row
CHUNK = 1250                       # zero-tile cols (5,000 B per row)
N_CHUNK = COLS // CHUNK            # 16 DMA writes per core

_cache: dict = {}


def _build() -> bass.Bass:
    nc = bass.Bass()
    out = nc.declare_dram_parameter("out", (P, COLS), mybir.dt.float32, isOutput=True)

    with (
        nc.sbuf_tensor([P, CHUNK], mybir.dt.float32) as zt,
        nc.semaphore("zsem") as zsem,
        nc.semaphore("dsem") as dsem,
        nc.Block() as block,
    ):

        @block.vector
        def _(vector):
            vector.memset(zt[:], 0.0).then_inc(zsem, 1)

        @block.sync
        def _(sync):
            sync.wait_ge(zsem, 1)
            for i in range(N_CHUNK):
                sync.dma_start(out[:, i * CHUNK : (i + 1) * CHUNK], zt[:]).then_inc(
                    dsem, 16
                )
            sync.wait_ge(dsem, 16 * N_CHUNK)

    return nc


def get_nc() -> bass.Bass:
    nc = _cache.get("nc")
    if nc is None:
        nc = _cache["nc"] = _build()
    return nc


def kernel(x, dw_weight, pw_weight, gamma, beta):
    assert x.shape == (B, C_IN, L), x.shape
    nc = get_nc()
    res = run_bass_kernel_spmd(
        nc, [dict() for _ in range(N_CORES)], core_ids=list(range(N_CORES))
    )
    shards = [r["out"].reshape(B_LOC, C_OUT, L) for r in res.results]
    out = np.ascontiguousarray(np.concatenate(shards, axis=0))
    reg_loss = np.float32(0.01) * np.float32(out.mean(dtype=np.float64))
    return out, reg_loss


# revision 2
# speedup vs baseline: 1.0960x; 1.0960x over previous
"""Trainium2 kernel for nn_LocalPatternExtractor (binary-weight depthwise+pointwise
conv -> BatchNorm -> quantized LIF over 4 timesteps).

Forward-pass analysis
---------------------
The reference quantizes the membrane potential with
    step = THRESHOLD / 2**(POT_BITS-1) = 1/128
    q    = clip(round(v/step), -128, 127) * step
so after quantization  mem <= 127/128 = 0.9921875 < THRESHOLD (=1.0), with
f32 STE round-off bounded by ~|v|*2^-24 << 1/128.  Hence `mem >= THRESHOLD`
is false for every element at every timestep, no spike ever fires, and the
forward output is identically
    out      = zeros((B, C_out, L), float32)
    reg_loss = SPIKE_REG * mean(out) = 0.0
for *all* finite inputs (verified empirically against the jax reference for
several seeds and 10x-scaled inputs).  The optimal kernel therefore reduces
to materializing the zero output at HBM write roofline.

Sharding: pure data parallel over the batch dim (16 -> 2 per core on 8
cores); each core zero-fills its own (2, 256, 5000) f32 output shard
(10.24 MB), which the host concatenates.
"""

import numpy as np

import concourse.bass as bass
import concourse.mybir as mybir
from concourse.bass_utils import run_bass_kernel_spmd

N_CORES = 8
B, C_IN, L = 16, 12, 5000
C_OUT = 256
K = 3

B_LOC = B // N_CORES               # 2 batches per core
OUT_ELEMS = B_LOC * C_OUT * L      # 2,560,000 f32 per core (10.24 MB)
P = 128                            # SBUF partitions
COLS = OUT_ELEMS // P              # 20,000 f32 per partition row

# Memset pieces (cols).  First pieces are small so the first DMA can launch
# early; each piece is one DMA, alternating between the two HWDGE queues
# (sync + scalar) so the 16 SDMA engines always have descriptors queued.
PIECES = [1250, 1250, 2500, 2500, 2500, 2500, 2500, 2500, 2500]
assert sum(PIECES) == COLS

_cache: dict = {}


def _build() -> bass.Bass:
    nc = bass.Bass()
    out = nc.declare_dram_parameter("out", (P, COLS), mybir.dt.float32, isOutput=True)

    starts = [sum(PIECES[:i]) for i in range(len(PIECES))]
    n_dma = len(PIECES)

    with (
        nc.sbuf_tensor([P, COLS], mybir.dt.float32) as zt,
        nc.semaphore("msem") as msem,
        nc.semaphore("dsem") as dsem,
        nc.Block() as block,
    ):

        @block.vector
        def _(vector):
            for s, w in zip(starts, PIECES):
                vector.memset(zt[:, s : s + w], 0.0).then_inc(msem, 1)

        @block.sync
        def _(sync):
            for c in range(0, n_dma, 2):
                s, w = starts[c], PIECES[c]
                sync.wait_ge(msem, c + 1)
                sync.dma_start(out[:, s : s + w], zt[:, s : s + w]).then_inc(dsem, 16)
            sync.wait_ge(dsem, 16 * n_dma)

        @block.scalar
        def _(scalar):
            for c in range(1, n_dma, 2):
                s, w = starts[c], PIECES[c]
                scalar.wait_ge(msem, c + 1)
                scalar.dma_start(out[:, s : s + w], zt[:, s : s + w]).then_inc(
                    dsem, 16
                )

    return nc


def get_nc() -> bass.Bass:
    nc = _cache.get("nc")
    if nc is None:
        nc = _cache["nc"] = _build()
    return nc


def kernel(x, dw_weight, pw_weight, gamma, beta):
    assert x.shape == (B, C_IN, L), x.shape
    nc = get_nc()
    res = run_bass_kernel_spmd(
        nc, [dict() for _ in range(N_CORES)], core_ids=list(range(N_CORES))
    )
    shards = [r["out"].reshape(B_LOC, C_OUT, L) for r in res.results]
    out = np.ascontiguousarray(np.concatenate(shards, axis=0))
    reg_loss = np.float32(0.01) * np.float32(out.mean(dtype=np.float64))
    return out, reg_loss
